# revision 1
# baseline (speedup 1.0000x reference)
"""Trainium2 Bass kernel for nn_CNN_symmetry (dense_cnn).

Strategy:
  * Pure data parallelism: batch B=32768 sharded across 8 NeuronCores (4096 each).
  * Host-side (numpy): build the 144x144 banded conv matrices from the tiny
    13x4 weights, pre-transpose them into lhsT blocks; rearrange dots into
    [pixel, batch] int32 planes (main 128 pixels + tail 16 pixels).
  * Device: convs as TensorE matmuls (bf16 in, fp32 PSUM), elementwise in bf16
    on VectorE/ScalarE/GpSimd with PSUM-fused evacuations.

Algebraic restructuring (validated vs reference in fp64):
    e=[x==0], m_c=[x==c];  C_sum=C_each+C_ne, C_epl=C_emp+C_ne, T=C_ne@1
    ne_e=C_ne@e ; t0=T-ne_e ; Ew'=C_epl@e-T (=E-t0) ; ie=1-e
    NECn_c = C_ne@m_c - t0        (= -NEC_c)
    s0 = sum_c m_c*(C_sum@m_c + Ew')
    all_v = sigmoid(s0);  Ebar = ie*(Ew'+t0)
    2x: g_c=-NECn_c*all_v; s += Ebar + sum_c m_c*(C_ne2@g_c); all_v=tanh(s/2)
    out = lrelu(lrelu(all_v@W1')@W2'+b2)@W3'+b3
"""

import os
import sys
from contextlib import ExitStack

import numpy as np

sys.path.insert(0, "/opt/trn_rl_repo")
os.environ.setdefault("MYCRO_LOCAL_CACHE", "1")

import ml_dtypes  # noqa: E402

import concourse.bass as bass  # noqa: E402
import concourse.bacc as bacc  # noqa: E402
import concourse.tile as tile  # noqa: E402
from concourse import mybir  # noqa: E402

V, H, B = 14, 12, 32768
NK, CT = 5, 4
HALF, FULL = 6, 13
NPIX, NSQ = 144, 100
NCORES = 8
BC = B // NCORES          # 4096 per core
CHUNK = 1024              # batch chunk processed per pipeline pass
NCHUNK = BC // CHUNK
PM, PT = 128, 16          # main/tail pixel split (i-major order p = i*12 + j)

BF16 = mybir.dt.bfloat16
F32 = mybir.dt.float32
I32 = mybir.dt.int32
AF = mybir.ActivationFunctionType
ALU = mybir.AluOpType


# ---------------------------------------------------------------- host prep

def _build_K(W):
    Wa = np.abs(np.asarray(W, np.float64))
    K = np.zeros((FULL, FULL))
    K[:, HALF:HALF + CT] = Wa
    K[:, HALF - CT + 1:HALF + 1] = Wa[:, ::-1]
    return K


def _band(K):
    C = np.zeros((NPIX, NPIX))
    for i in range(12):
        for j in range(12):
            for i2 in range(12):
                for j2 in range(12):
                    di, dj = i2 - i + HALF, j2 - j + HALF
                    if 0 <= di < FULL and 0 <= dj < FULL:
                        C[i * 12 + j, i2 * 12 + j2] = K[di, dj]
    return C


def _lhsT_blocks(C):
    """C (out,in) -> lhsT = C.T blocks: mm [128,128], tm [16,128], mt [128,16], tt [16,16]."""
    L = C.T.astype(np.float64)  # [in=K, out=M]
    return {
        "mm": L[:PM, :PM], "tm": L[PM:, :PM],
        "mt": L[:PM, PM:], "tt": L[PM:, PM:],
    }


def build_consts(w_each, w_not_each, w_not_each_2nd, w_empty, W1, W2, b2, W3, b3):
    C_each = _band(_build_K(w_each))
    C_ne = _band(_build_K(w_not_each))
    C_ne2 = _band(_build_K(w_not_each_2nd))
    C_emp = _band(_build_K(w_empty))
    C_sum = C_each + C_ne
    C_epl = C_emp + C_ne
    T = C_ne @ np.ones(NPIX)

    bf = lambda a: np.ascontiguousarray(np.asarray(a), dtype=ml_dtypes.bfloat16)
    f32 = lambda a: np.ascontiguousarray(np.asarray(a), dtype=np.float32)

    consts = {}
    for name, C in (("csum", C_sum), ("cne", C_ne), ("cepl", C_epl), ("cne2", C_ne2)):
        for bk, arr in _lhsT_blocks(C).items():
            consts[f"{name}_{bk}"] = bf(arr)
    consts["t_m"] = f32(T[:PM].reshape(PM, 1))
    consts["t_t"] = f32(T[PM:].reshape(PT, 1))
    W1T = np.asarray(W1, np.float64).T        # [144, 100]
    consts["w1_m"] = bf(W1T[:PM])
    consts["w1_t"] = bf(W1T[PM:])
    consts["w2"] = bf(np.asarray(W2, np.float64).T)   # [100, 100]
    consts["w3"] = bf(np.asarray(W3, np.float64).T)   # [100, 1]
    consts["b2"] = f32(np.asarray(b2).reshape(NSQ, 1))
    consts["b3"] = f32(np.asarray(b3).reshape(1, 1))
    return consts


CONST_SPECS = (
    [(f"{n}_{bk}", ([PM, PM] if bk == "mm" else [PT, PM] if bk == "tm"
                    else [PM, PT] if bk == "mt" else [PT, PT]), BF16)
     for n in ("csum", "cne", "cepl", "cne2") for bk in ("mm", "tm", "mt", "tt")]
    + [("t_m", [PM, 1], F32), ("t_t", [PT, 1], F32),
       ("w1_m", [PM, NSQ], BF16), ("w1_t", [PT, NSQ], BF16),
       ("w2", [NSQ, NSQ], BF16), ("w3", [NSQ, 1], BF16), ("b2", [NSQ, 1], F32), ("b3", [1, 1], F32)]
)


# ---------------------------------------------------------------- device kernel

def emit_kernel(nc, bc, chunk):
    nchunk = bc // chunk
    xm_d = nc.dram_tensor("xm", [PM, bc], I32, kind="ExternalInput")
    xt_d = nc.dram_tensor("xt", [PT, bc], I32, kind="ExternalInput")
    out_d = nc.dram_tensor("out", [1, bc], F32, kind="ExternalOutput")
    const_d = {n: nc.dram_tensor(n, shp, dt, kind="ExternalInput")
               for n, shp, dt in CONST_SPECS}

    with tile.TileContext(nc) as tc, ExitStack() as ctx:
        cpool = ctx.enter_context(tc.tile_pool(name="consts", bufs=1))
        xpool = ctx.enter_context(tc.tile_pool(name="x", bufs=2))
        mpool = ctx.enter_context(tc.tile_pool(name="masks", bufs=1))
        npool = ctx.enter_context(tc.tile_pool(name="necn", bufs=1))
        spool = ctx.enter_context(tc.tile_pool(name="smisc", bufs=1))
        gpool = ctx.enter_context(tc.tile_pool(name="g", bufs=3))
        ppool = ctx.enter_context(tc.tile_pool(name="ps", bufs=2, space="PSUM"))
        tpool = ctx.enter_context(tc.tile_pool(name="pst", bufs=2, space="PSUM"))
        opool = ctx.enter_context(tc.tile_pool(name="outs", bufs=2))

        # ACT table-set warmup: the first Activation carries the implicit
        # table-load sync; give it zero data deps so its wait budget fits.
        warm = cpool.tile([1, 1], F32, tag="warm", name="warm")
        nc.vector.memset(warm[:], 0.0)
        nc.scalar.activation(warm[:], warm[:], AF.Copy)
        nc.scalar.activation(warm[:], warm[:], AF.Sigmoid)
        nc.scalar.activation(warm[:], warm[:], AF.Tanh)

        # load constants once
        C = {}
        for n, shp, dt in CONST_SPECS:
            t = cpool.tile(shp, dt, tag=n, name=n)
            nc.gpsimd.dma_start(t[:], const_d[n][:])
            C[n] = t

        xm_f = xpool.tile([PM, bc], I32, tag="xm", name="xm_t", bufs=1)
        xt_f = xpool.tile([PT, bc], I32, tag="xt", name="xt_t", bufs=1)
        nc.gpsimd.dma_start(xm_f[:], xm_d[:])
        nc.gpsimd.dma_start(xt_f[:], xt_d[:])

        for ck in range(nchunk):
            c0 = ck * chunk
            xm = xm_f[:, c0:c0 + chunk]
            xt = xt_f[:, c0:c0 + chunk]

            # --- masks (bf16 0/1): e, m_1..m_5 --------------------------------
            masks = []  # (main, tail) pairs; masks[0] is e
            for c in range(NK + 1):
                mm_ = mpool.tile([PM, chunk], BF16, tag=f"mkm{c}", name=f"mkm{c}")
                mt_ = mpool.tile([PT, chunk], BF16, tag=f"mkt{c}", name=f"mkt{c}")
                nc.vector.tensor_scalar(mm_[:], xm[:], c, None, ALU.is_equal)
                nc.vector.tensor_scalar(mt_[:], xt[:], c, None, ALU.is_equal)
                masks.append((mm_, mt_))

            def conv(mat, rhs_m, rhs_t, evac, n_banks=2):
                """conv with matrix `mat` over [rhs_m;rhs_t], call evac(ps, part, lo)
                with ps = psum tile [128, 512*n_banks] or [16, 512*n_banks]."""
                w = min(512 * n_banks, chunk)
                for lo in range(0, chunk, w):
                    ps = ppool.tile([PM, w], F32, tag="psm", name="psm")
                    for nn in range(0, w, 512):
                        nc.tensor.matmul(ps[:, nn:nn + 512], C[f"{mat}_mm"][:],
                                         rhs_m[:, lo + nn:lo + nn + 512],
                                         start=True, stop=False)
                    for nn in range(0, w, 512):
                        nc.tensor.matmul(ps[:, nn:nn + 512], C[f"{mat}_tm"][:],
                                         rhs_t[:, lo + nn:lo + nn + 512],
                                         start=False, stop=True)
                    evac(ps, "m", lo, w)
                for lo in range(0, chunk, w):
                    ps = tpool.tile([PT, w], F32, tag="pst", name="pst")
                    for nn in range(0, w, 512):
                        nc.tensor.matmul(ps[:, nn:nn + 512], C[f"{mat}_mt"][:],
                                         rhs_m[:, lo + nn:lo + nn + 512],
                                         start=True, stop=False)
                    for nn in range(0, w, 512):
                        nc.tensor.matmul(ps[:, nn:nn + 512], C[f"{mat}_tt"][:],
                                         rhs_t[:, lo + nn:lo + nn + 512],
                                         start=False, stop=True)
                    evac(ps, "t", lo, w)

            def tiles(tag, dt=BF16, pool=spool):
                return (pool.tile([PM, chunk], dt, tag=tag + "m", name=tag + "m"),
                        pool.tile([PT, chunk], dt, tag=tag + "t", name=tag + "t"))

            def part(pair, p, lo, w):
                t = pair[0] if p == "m" else pair[1]
                return t[:, lo:lo + w]

            W_ = min(1024, chunk)  # evac granularity (psum tile free size)

            # --- e-convs: t0 = T - C_ne@e ; Ew' = C_epl@e - T  (ScalarE) ------
            t0 = tiles("t0")
            ew = tiles("ew")

            def ev_t0(ps, p, lo, w):
                tv = C["t_m"] if p == "m" else C["t_t"]
                nc.scalar.activation(part(t0, p, lo, w), ps[:],
                                     AF.Identity, bias=tv[:], scale=-1.0)

            conv("cne", masks[0][0], masks[0][1], ev_t0)

            # Ew' needs bias = -T: negate via vector tensor_scalar instead
            def ev_ew2(ps, p, lo, w):
                tv = C["t_m"] if p == "m" else C["t_t"]
                nc.vector.tensor_scalar(part(ew, p, lo, w), ps[:],
                                        tv[:], None, ALU.subtract)

            conv("cepl", masks[0][0], masks[0][1], ev_ew2)

            # ie = 1 - e ; Ebar = ie * (Ew' + t0) ------------------------------
            ie = tiles("ie")
            nc.vector.tensor_scalar(ie[0][:], xm[:], 0, None, ALU.not_equal)
            nc.vector.tensor_scalar(ie[1][:], xt[:], 0, None, ALU.not_equal)
            ebar = tiles("ebar")
            for r in range(2):
                nc.vector.tensor_tensor(ebar[r][:], ew[r][:], t0[r][:], ALU.add)
                nc.vector.tensor_tensor(ebar[r][:], ebar[r][:], ie[r][:], ALU.mult)

            # --- per-color: NECn_c = C_ne@m_c - t0 ;
            #     s0 = sum_c m_c*S_c + ie*Ew'  (masks disjoint -> predicated sel)
            necn = [tiles(f"necn{c}", pool=npool) for c in range(NK)]
            s = tiles("s")

            def prod_scratch(p, lo, w):
                t = (spool.tile([PM, chunk], BF16, tag="prodm", name="prodm",
                                bufs=2) if p == "m" else
                     spool.tile([PT, chunk], BF16, tag="prodt", name="prodt",
                                bufs=2))
                return t[:, lo:lo + w]

            iew = tiles("iew")
            for r in range(2):
                nc.vector.tensor_tensor(iew[r][:], ie[r][:], ew[r][:], ALU.mult)

            for c in range(NK):
                def ev_necn(ps, p, lo, w, c=c):
                    scr = prod_scratch(p, lo, w)
                    nc.scalar.activation(scr, ps[:], AF.Copy)
                    nc.vector.tensor_tensor(part(necn[c], p, lo, w), scr,
                                            part(t0, p, lo, w), ALU.subtract)
                conv("cne", masks[c + 1][0], masks[c + 1][1], ev_necn)

                def ev_s(ps, p, lo, w, c=c):
                    # s += m_c * S_c  (Ew' part folded into iew once)
                    if c == 0:
                        nc.vector.tensor_tensor(part(s, p, lo, w), ps[:],
                                                part(masks[1], p, lo, w), ALU.mult)
                    else:
                        scr = prod_scratch(p, lo, w)
                        nc.vector.tensor_tensor(scr, ps[:],
                                                part(masks[c + 1], p, lo, w),
                                                ALU.mult)
                        sd = part(s, p, lo, w)
                        nc.vector.tensor_tensor(sd, sd, scr, ALU.add)
                conv("csum", masks[c + 1][0], masks[c + 1][1], ev_s)
            for r in range(2):
                nc.vector.tensor_tensor(s[r][:], s[r][:], iew[r][:], ALU.add)

            allv = tiles("allv")
            for r in range(2):
                nc.scalar.activation(allv[r][:], s[r][:], AF.Sigmoid)

            # --- depth loop x2 ------------------------------------------------
            for it in range(2):
                acc = tiles("acc")
                for c in range(NK):
                    g = (gpool.tile([PM, chunk], BF16, tag="gm", name="gm"),
                         gpool.tile([PT, chunk], BF16, tag="gt", name="gt"))
                    for r in range(2):
                        # g = NECn*allv = -g_true; conv linear -> s -= sel below
                        nc.vector.tensor_tensor(g[r][:], necn[c][r][:],
                                                allv[r][:], ALU.mult)

                    def ev_acc(ps, p, lo, w, c=c, acc=acc):
                        # acc += m_c * (C@g) ; g = NECn*allv = -g_true -> s -= acc
                        if c == 0:
                            nc.vector.tensor_tensor(part(acc, p, lo, w), ps[:],
                                                    part(masks[1], p, lo, w),
                                                    ALU.mult)
                        else:
                            scr = prod_scratch(p, lo, w)
                            nc.vector.tensor_tensor(scr, ps[:],
                                                    part(masks[c + 1], p, lo, w),
                                                    ALU.mult)
                            ad = part(acc, p, lo, w)
                            nc.vector.tensor_tensor(ad, ad, scr, ALU.add)
                    conv("cne2", g[0], g[1], ev_acc)

                for r in range(2):
                    nc.vector.tensor_tensor(s[r][:], s[r][:], ebar[r][:], ALU.add)
                    nc.vector.tensor_tensor(s[r][:], s[r][:], acc[r][:], ALU.subtract)
                    nc.scalar.activation(allv[r][:], s[r][:], AF.Tanh, scale=0.5)

            # --- MLP ----------------------------------------------------------
            h1 = opool.tile([NSQ, chunk], BF16, tag="h1", name="h1")
            for lo in range(0, chunk, W_):
                ps = ppool.tile([PM, W_], F32, tag="psm", name="psm")
                for nn in range(0, W_, 512):
                    sl = slice(lo + nn, lo + nn + 512)
                    nc.tensor.matmul(ps[:NSQ, nn:nn + 512], C["w1_m"][:],
                                     allv[0][:, sl], start=True, stop=False)
                    nc.tensor.matmul(ps[:NSQ, nn:nn + 512], C["w1_t"][:],
                                     allv[1][:, sl], start=False, stop=True)
                nc.scalar.activation(h1[:, lo:lo + W_], ps[:NSQ], AF.Copy)
                nc.vector.scalar_tensor_tensor(
                    h1[:, lo:lo + W_], ps[:NSQ], 0.2, h1[:, lo:lo + W_],
                    ALU.mult, ALU.max)
            h2 = opool.tile([NSQ, chunk], BF16, tag="h2", name="h2")
            for lo in range(0, chunk, W_):
                ps = ppool.tile([PM, W_], F32, tag="psm", name="psm")
                for nn in range(0, W_, 512):
                    sl = slice(lo + nn, lo + nn + 512)
                    nc.tensor.matmul(ps[:NSQ, nn:nn + 512], C["w2"][:],
                                     h1[:, sl], start=True, stop=True)
                nc.scalar.activation(h2[:, lo:lo + W_], ps[:NSQ], AF.Identity,
                                     bias=C["b2"][:])
                nc.vector.scalar_tensor_tensor(
                    h2[:, lo:lo + W_], h2[:, lo:lo + W_], 0.2,
                    h2[:, lo:lo + W_], ALU.mult, ALU.max)
            yout = opool.tile([1, chunk], F32, tag="yout", name="yout")
            for lo in range(0, chunk, W_):
                ps = tpool.tile([PT, W_], F32, tag="pst", name="pst")
                for nn in range(0, W_, 512):
                    sl = slice(lo + nn, lo + nn + 512)
                    nc.tensor.matmul(ps[:1, nn:nn + 512], C["w3"][:],
                                     h2[:, sl], start=True, stop=True)
                nc.scalar.activation(yout[:, lo:lo + W_], ps[:1], AF.Identity,
                                     bias=C["b3"][:])
            nc.gpsimd.dma_start(out_d[:, c0:c0 + chunk], yout[:])

    return nc


# ---------------------------------------------------------------- entry point

def _prep_inputs(dots):
    """dots (14,12,B) int32 -> per-core xm [128, BC], xt [16, BC]."""
    x = np.ascontiguousarray(np.asarray(dots)[:12].reshape(NPIX, B), dtype=np.int32)
    xms, xts = [], []
    for k in range(NCORES):
        sl = x[:, k * BC:(k + 1) * BC]
        xms.append(np.ascontiguousarray(sl[:PM]))
        xts.append(np.ascontiguousarray(sl[PM:]))
    return xms, xts


def kernel(dots, w_each, w_not_each, w_not_each_2nd, w_empty, W1, W2, b2, W3, b3):
    from concourse.bass_utils import run_bass_kernel_spmd

    consts = build_consts(w_each, w_not_each, w_not_each_2nd, w_empty,
                          W1, W2, b2, W3, b3)
    xms, xts = _prep_inputs(dots)

    nc = bacc.Bacc()
    emit_kernel(nc, BC, CHUNK)
    nc.compile()

    in_maps = [dict(consts, xm=xms[k], xt=xts[k]) for k in range(NCORES)]
    res = run_bass_kernel_spmd(nc, in_maps, list(range(NCORES)))
    out = np.concatenate([np.asarray(r["out"]).reshape(BC) for r in res.results])
    return out.reshape(B, 1).astype(np.float32)


if __name__ == "__main__":
    rng = np.random.default_rng(0)
    ins = {
        "dots": rng.integers(0, 6, size=(V, H, B)).astype(np.int32),
        "w_each": rng.standard_normal((FULL, CT), dtype=np.float32) * 0.1,
        "w_not_each": rng.standard_normal((FULL, CT), dtype=np.float32) * 0.1,
        "w_not_each_2nd": rng.standard_normal((FULL, CT), dtype=np.float32) * 0.1,
        "w_empty": rng.standard_normal((FULL, CT), dtype=np.float32) * 0.1,
        "W1": rng.standard_normal((NSQ, NPIX), dtype=np.float32) * 0.2,
        "W2": rng.standard_normal((NSQ, NSQ), dtype=np.float32) * 0.2,
        "b2": rng.standard_normal(NSQ, dtype=np.float32) * 0.1,
        "W3": rng.standard_normal((1, NSQ), dtype=np.float32) * 0.2,
        "b3": rng.standard_normal(1, dtype=np.float32) * 0.1,
    }
    y = kernel(**ins)
    print("kernel out", y.shape, y[:4, 0])



# revision 9
# speedup vs baseline: 1.6198x; 1.6198x over previous
"""Trainium2 Bass kernel for nn_CNN_symmetry (dense_cnn).

Strategy v2:
  * Pure data parallelism: B=32768 sharded across 8 NeuronCores (4096 each).
  * Per core: 4 "dblocks" of 1024 batch cols. Main 128 pixels as [128, 1024]
    tiles; the 16 tail pixels of all 4 dblocks PACKED into one [128, 1024]
    tile at 32-stride partition slots (strip j = dblock j), so all tail
    elementwise work runs once per core instead of once per dblock.
  * Tail conv outputs land partition-packed in PSUM via tile_position
    col-strips (mt: (0,32j), tt: (32j,32j)); tail conv inputs feed via
    row-strips (tm: (32j,0)). Col-strip matmuls run concurrently on the PE.
  * Masked selects via copy_predicated reading PSUM directly (int16 views
    of bf16 masks). MLP lrelu/bias fused into ScalarE activations.

Algebra (same math as reference, restructured):
    e=[x==0], m_c=[x==c], ie=1-e; C_sum=C_each+C_ne; T=C_ne@1
    t0 = T - C_ne@e ; E0 = C_emp@e ; ew = E0 - t0 ; ebar = ie*E0
    necn_c = C_ne@m_c - t0
    s0 = sum_c m_c*(C_sum@m_c) + ie*ew ; allv = sigmoid(s0)
    2x: asel = sum_c m_c*(C_ne2@(necn_c*allv)); s += ebar - asel
        allv = tanh(s/2)
    out = lrelu(lrelu(allv@W1')@W2'+b2)@W3'+b3
"""

import os
import sys
from contextlib import ExitStack

import numpy as np

sys.path.insert(0, "/opt/trn_rl_repo")
os.environ.setdefault("MYCRO_LOCAL_CACHE", "1")

import ml_dtypes  # noqa: E402

import concourse.bass as bass  # noqa: E402
import concourse.bacc as bacc  # noqa: E402
import concourse.tile as tile  # noqa: E402
from concourse import mybir  # noqa: E402

V, H, B = 14, 12, 32768
NK, CT = 5, 4
HALF, FULL = 6, 13
NPIX, NSQ = 144, 100
NCORES = 8
BC = B // NCORES          # 4096 per core
DB = 1024                 # dblock width
NDB = BC // DB            # 4
PM, PT = 128, 16          # main/tail pixel split (i-major order p = i*12 + j)

BF16 = mybir.dt.bfloat16
F32 = mybir.dt.float32
I16 = mybir.dt.int16
AF = mybir.ActivationFunctionType
ALU = mybir.AluOpType


# ---------------------------------------------------------------- host prep

def _build_K(W):
    Wa = np.abs(np.asarray(W, np.float64))
    K = np.zeros((FULL, FULL))
    K[:, HALF:HALF + CT] = Wa
    K[:, HALF - CT + 1:HALF + 1] = Wa[:, ::-1]
    return K


def _band(K):
    C = np.zeros((NPIX, NPIX))
    for i in range(12):
        for j in range(12):
            for i2 in range(12):
                for j2 in range(12):
                    di, dj = i2 - i + HALF, j2 - j + HALF
                    if 0 <= di < FULL and 0 <= dj < FULL:
                        C[i * 12 + j, i2 * 12 + j2] = K[di, dj]
    return C


def _strip4(block16):
    """[16, W] -> [128, W] with copies at partition offsets 0/32/64/96."""
    W = block16.shape[1]
    out = np.zeros((PM, W), block16.dtype)
    for j in range(4):
        out[32 * j:32 * j + PT] = block16
    return out


def build_consts(w_each, w_not_each, w_not_each_2nd, w_empty, W1, W2, b2, W3, b3):
    C_each = _band(_build_K(w_each))
    C_ne = _band(_build_K(w_not_each))
    C_ne2 = _band(_build_K(w_not_each_2nd))
    C_emp = _band(_build_K(w_empty))
    C_sum = C_each + C_ne
    T = C_ne @ np.ones(NPIX)

    bf = lambda a: np.ascontiguousarray(np.asarray(a), dtype=ml_dtypes.bfloat16)
    f32 = lambda a: np.ascontiguousarray(np.asarray(a), dtype=np.float32)

    consts = {}
    for name, C in (("csum", C_sum), ("cne", C_ne), ("cemp", C_emp), ("cne2", C_ne2)):
        L = C.T  # lhsT [in, out]
        consts[f"{name}_mm"] = bf(L[:PM, :PM])
        consts[f"{name}_tm4"] = bf(_strip4(L[PM:, :PM]))       # [128,128]
        consts[f"{name}_mt"] = bf(L[:PM, PM:])                 # [128,16]
        consts[f"{name}_tt4"] = bf(_strip4(L[PM:, PM:]))       # [128,16]
    consts["t_m"] = f32(T[:PM].reshape(PM, 1))
    consts["t_t4"] = f32(_strip4(T[PM:].reshape(PT, 1).astype(np.float32)))
    W1T = np.asarray(W1, np.float64).T        # [144, 100]
    consts["w1_m"] = bf(W1T[:PM])
    consts["w1_t4"] = bf(_strip4(W1T[PM:].astype(np.float64)))  # [128, 100]
    consts["w2"] = bf(np.asarray(W2, np.float64).T)   # [100, 100]
    consts["w3"] = bf(np.asarray(W3, np.float64).T)   # [100, 1]
    consts["b2"] = f32(np.asarray(b2).reshape(NSQ, 1))
    consts["b3"] = f32(np.asarray(b3).reshape(1, 1))
    return consts


CONST_SPECS = (
    [(f"{n}_{bk}", shp, BF16)
     for n in ("csum", "cne", "cemp", "cne2")
     for bk, shp in (("mm", [PM, PM]), ("tm4", [PM, PM]),
                     ("mt", [PM, PT]), ("tt4", [PM, PT]))]
    + [("t_m", [PM, 1], F32), ("t_t4", [PM, 1], F32),
       ("w1_m", [PM, NSQ], BF16), ("w1_t4", [PM, NSQ], BF16),
       ("w2", [NSQ, NSQ], BF16), ("w3", [NSQ, 1], BF16),
       ("b2", [NSQ, 1], F32), ("b3", [1, 1], F32)]
)


# ---------------------------------------------------------------- device kernel

def emit_kernel(nc):
    xm_d = nc.dram_tensor("xm", [PM, BC], BF16, kind="ExternalInput")
    xt_d = nc.dram_tensor("xt4", [PM, DB], BF16, kind="ExternalInput")
    out_d = nc.dram_tensor("out", [1, BC], F32, kind="ExternalOutput")
    const_d = {n: nc.dram_tensor(n, shp, dt, kind="ExternalInput")
               for n, shp, dt in CONST_SPECS}

    with tile.TileContext(nc) as tc, ExitStack() as ctx:
        cpool = ctx.enter_context(tc.tile_pool(name="consts", bufs=1))
        xpool = ctx.enter_context(tc.tile_pool(name="x", bufs=1))
        qpool = ctx.enter_context(tc.tile_pool(name="perq", bufs=1))
        shpool = ctx.enter_context(tc.tile_pool(name="shared", bufs=2))
        npool = ctx.enter_context(tc.tile_pool(name="nscr", bufs=3))
        gpool = ctx.enter_context(tc.tile_pool(name="g", bufs=1))
        opool = ctx.enter_context(tc.tile_pool(name="outs", bufs=1))
        ppool = ctx.enter_context(tc.tile_pool(name="ps", bufs=1, space="PSUM"))
        tpool = ctx.enter_context(tc.tile_pool(name="pst", bufs=1, space="PSUM"))

        # ACT table warmup with no data deps
        warm = cpool.tile([1, 1], F32, tag="warm", name="warm")
        nc.vector.memset(warm[:], 0.0)
        nc.scalar.activation(warm[:], warm[:], AF.Copy)
        nc.scalar.activation(warm[:], warm[:], AF.Sigmoid)
        nc.scalar.activation(warm[:], warm[:], AF.Tanh)

        C = {}
        for n, shp, dt in CONST_SPECS:
            t = cpool.tile(shp, dt, tag=n, name=n)
            nc.gpsimd.dma_start(t[:], const_d[n][:])
            C[n] = t

        xm = xpool.tile([PM, BC], BF16, tag="xm", name="xm_t")
        xt = xpool.tile([PM, DB], BF16, tag="xt", name="xt_t")
        nc.gpsimd.dma_start(xm[:], xm_d[:])
        nc.gpsimd.dma_start(xt[:], xt_d[:])

        NQ = NDB + 1   # 4 main dblocks + 1 tail set (index NDB)

        def xq(q):
            # x source for set q: main slice or tail pack
            return xm[:, q * DB:(q + 1) * DB] if q < NDB else xt[:]

        # ---- persistent per-set tiles -------------------------------------
        mstk = [qpool.tile([PM, NK * DB], BF16, tag=f"mstk{q}", name=f"mstk{q}")
                for q in range(NQ)]          # masks m_1..m_5 stacked on free dim
        necn = [qpool.tile([PM, NK * DB], BF16, tag=f"necn{q}", name=f"necn{q}")
                for q in range(NQ)]
        s_t = [qpool.tile([PM, DB], BF16, tag=f"s{q}", name=f"s{q}")
               for q in range(NQ)]
        allv = [qpool.tile([PM, DB], BF16, tag=f"allv{q}", name=f"allv{q}")
                for q in range(NQ)]
        ebar = [qpool.tile([PM, DB], BF16, tag=f"ebar{q}", name=f"ebar{q}")
                for q in range(NQ)]
        sel = [qpool.tile([PM, DB], BF16, tag=f"sel{q}", name=f"sel{q}")
               for q in range(NQ)]
        t0_t = [qpool.tile([PM, DB], BF16, tag=f"t0{q}", name=f"t0{q}")
                for q in range(NQ)]

        def msl(q, c):
            return mstk[q][:, (c - 1) * DB:c * DB]

        def mint(q, c):
            return msl(q, c).bitcast(I16)

        # ---- conv stage helper --------------------------------------------
        def conv_stage(mat, rhs_main, rhs_tail, evac_main, evac_tail):
            """rhs_main(j)->AP [128, DB]; rhs_tail->AP [128, DB] pack (strips).
            evac_main(j, ps), evac_tail(ps)."""
            for j in range(NDB):
                ps = ppool.tile([PM, DB], F32, tag=f"pm{j % 3}", name=f"pm{j % 3}")
                rm = rhs_main(j)
                for nn in (0, 512):
                    nc.tensor.matmul(ps[:, nn:nn + 512], C[f"{mat}_mm"][:],
                                     rm[:, nn:nn + 512], start=True, stop=False)
                for nn in (0, 512):
                    nc.tensor.matmul(ps[:, nn:nn + 512],
                                     C[f"{mat}_tm4"][32 * j:32 * j + PT, :],
                                     rhs_tail[32 * j:32 * j + PT, nn:nn + 512],
                                     start=False, stop=True,
                                     tile_position=(32 * j, 0),
                                     skip_group_check=True)
                evac_main(j, ps)
            pst = tpool.tile([PM, DB], F32, tag="pt", name="pt")
            for nn in (0, 512):
                for j in range(NDB):
                    rm = rhs_main(j)
                    nc.tensor.matmul(pst[32 * j:32 * j + PT, nn:nn + 512],
                                     C[f"{mat}_mt"][:],
                                     rm[:, nn:nn + 512], start=True, stop=False,
                                     tile_position=(0, 32 * j),
                                     skip_group_check=True)
                for j in range(NDB):
                    nc.tensor.matmul(pst[32 * j:32 * j + PT, nn:nn + 512],
                                     C[f"{mat}_tt4"][32 * j:32 * j + PT, :],
                                     rhs_tail[32 * j:32 * j + PT, nn:nn + 512],
                                     start=False, stop=True,
                                     tile_position=(32 * j, 32 * j),
                                     skip_group_check=True)
            evac_tail(pst)

        # ---- phase A: masks ----------------------------------------------
        e_t = [qpool.tile([PM, DB], BF16, tag=f"e{q}", name=f"e{q}")
               for q in range(NQ)]
        for q in range(NQ):
            x = xq(q)
            nc.vector.tensor_scalar(e_t[q][:], x, 0, None, ALU.is_equal)
            for c in range(1, NK + 1):
                nc.vector.tensor_scalar(msl(q, c), x, c, None, ALU.is_equal)
            nc.gpsimd.memset(sel[q][:], 0.0)

        # ---- phase B: e-convs --------------------------------------------
        def ev_t0_m(j, ps):
            nc.scalar.activation(t0_t[j][:], ps[:], AF.Identity,
                                 bias=C["t_m"][:], scale=-1.0)

        def ev_t0_t(ps):
            nc.scalar.activation(t0_t[NDB][:], ps[:], AF.Identity,
                                 bias=C["t_t4"][:], scale=-1.0)

        conv_stage("cne", lambda j: e_t[j][:], e_t[NDB][:], ev_t0_m, ev_t0_t)

        def ev_e0(q, ps):
            e0 = shpool.tile([PM, DB], BF16, tag="e0", name=f"e0{q}")
            ie = shpool.tile([PM, DB], BF16, tag="ie", name=f"ie{q}")
            nc.scalar.activation(e0[:], ps[:], AF.Copy)
            nc.vector.tensor_scalar(ie[:], xq(q), 0, None, ALU.not_equal)
            # ebar = ie*E0 ; ew = E0-t0 (in place) ; s = ie*ew
            nc.vector.tensor_tensor(ebar[q][:], ie[:], e0[:], ALU.mult)
            nc.vector.tensor_tensor(e0[:], e0[:], t0_t[q][:], ALU.subtract)
            nc.vector.tensor_tensor(s_t[q][:], ie[:], e0[:], ALU.mult)

        conv_stage("cemp", lambda j: e_t[j][:], e_t[NDB][:],
                   lambda j, ps: ev_e0(j, ps), lambda ps: ev_e0(NDB, ps))

        # ---- phase C: per-color convs ------------------------------------
        for c in range(1, NK + 1):
            def ev_n(q, ps, c=c):
                nscr = npool.tile([PM, DB], BF16, tag="n", name=f"n{q}")
                nc.scalar.activation(nscr[:], ps[:], AF.Copy)
                nc.vector.tensor_tensor(necn[q][:, (c - 1) * DB:c * DB],
                                        nscr[:], t0_t[q][:], ALU.subtract)

            conv_stage("cne", lambda j, c=c: msl(j, c), msl(NDB, c),
                       lambda j, ps: ev_n(j, ps), lambda ps: ev_n(NDB, ps))

            def ev_u(q, ps, c=c):
                nc.vector.copy_predicated(sel[q][:], mint(q, c), ps[:])

            conv_stage("csum", lambda j, c=c: msl(j, c), msl(NDB, c),
                       lambda j, ps: ev_u(j, ps), lambda ps: ev_u(NDB, ps))

        # ---- s0 / sigmoid -------------------------------------------------
        for q in range(NQ):
            nc.vector.tensor_tensor(s_t[q][:], s_t[q][:], sel[q][:], ALU.add)
            nc.scalar.activation(allv[q][:], s_t[q][:], AF.Sigmoid)

        # ---- depth loop x2 ------------------------------------------------
        for it in range(2):
            for q in range(NQ):
                nc.gpsimd.memset(sel[q][:], 0.0)
            for c in range(1, NK + 1):
                gt = [gpool.tile([PM, DB], BF16, tag=f"g{q}", name=f"g{q}")
                      for q in range(NQ)]
                for q in range(NQ):
                    nc.vector.tensor_tensor(gt[q][:],
                                            necn[q][:, (c - 1) * DB:c * DB],
                                            allv[q][:], ALU.mult)

                def ev_a(q, ps, c=c):
                    nc.vector.copy_predicated(sel[q][:], mint(q, c), ps[:])

                conv_stage("cne2", lambda j: gt[j][:], gt[NDB][:],
                           lambda j, ps: ev_a(j, ps), lambda ps: ev_a(NDB, ps))
            for q in range(NQ):
                nc.vector.tensor_tensor(s_t[q][:], s_t[q][:], ebar[q][:], ALU.add)
                nc.vector.tensor_tensor(s_t[q][:], s_t[q][:], sel[q][:], ALU.subtract)
                nc.scalar.activation(allv[q][:], s_t[q][:], AF.Tanh, scale=0.5)

        # ---- MLP ----------------------------------------------------------
        for j in range(NDB):
            ps1 = ppool.tile([PM, DB], F32, tag=f"pm{j % 3}", name=f"mlp1_{j}")
            for nn in (0, 512):
                nc.tensor.matmul(ps1[:NSQ, nn:nn + 512], C["w1_m"][:],
                                 allv[j][:, nn:nn + 512], start=True, stop=False)
                nc.tensor.matmul(ps1[:NSQ, nn:nn + 512],
                                 C["w1_t4"][32 * j:32 * j + PT, :],
                                 allv[NDB][32 * j:32 * j + PT, nn:nn + 512],
                                 start=False, stop=True,
                                 tile_position=(32 * j, 0),
                                 skip_group_check=True)
            h1 = opool.tile([NSQ, DB], BF16, tag="h1", name="h1")
            nc.scalar.activation(h1[:], ps1[:NSQ], AF.Copy)
            nc.vector.scalar_tensor_tensor(h1[:], ps1[:NSQ], 0.2, h1[:],
                                           ALU.mult, ALU.max)
            ps2 = tpool.tile([PM, DB], F32, tag="pt", name=f"mlp2_{j}")
            for nn in (0, 512):
                nc.tensor.matmul(ps2[:NSQ, nn:nn + 512], C["w2"][:],
                                 h1[:, nn:nn + 512], start=True, stop=True)
            h2 = opool.tile([NSQ, DB], BF16, tag="h2", name="h2")
            nc.scalar.activation(h2[:], ps2[:NSQ], AF.Identity, bias=C["b2"][:])
            nc.vector.scalar_tensor_tensor(h2[:], h2[:], 0.2, h2[:],
                                           ALU.mult, ALU.max)
            ps3 = ppool.tile([PM, DB], F32, tag=f"pm{(j + 1) % 3}", name=f"mlp3_{j}")
            for nn in (0, 512):
                nc.tensor.matmul(ps3[:1, nn:nn + 512], C["w3"][:],
                                 h2[:, nn:nn + 512], start=True, stop=True)
            yout = opool.tile([1, DB], F32, tag="yout", name="yout")
            nc.scalar.activation(yout[:], ps3[:1], AF.Identity, bias=C["b3"][:])
            nc.gpsimd.dma_start(out_d[:, j * DB:(j + 1) * DB], yout[:])

    return nc


# ---------------------------------------------------------------- entry point

def _prep_inputs(dots):
    """dots (14,12,B) int32 -> per-core xm [128, BC] bf16, xt4 [128, DB] bf16."""
    x = np.asarray(dots)[:12].reshape(NPIX, B).astype(ml_dtypes.bfloat16)
    xms, xts = [], []
    for k in range(NCORES):
        sl = x[:, k * BC:(k + 1) * BC]
        xms.append(np.ascontiguousarray(sl[:PM]))
        tail = sl[PM:]                       # [16, 4096]
        pack = np.zeros((PM, DB), ml_dtypes.bfloat16)
        for j in range(NDB):
            pack[32 * j:32 * j + PT] = tail[:, j * DB:(j + 1) * DB]
        xts.append(pack)
    return xms, xts


def kernel(dots, w_each, w_not_each, w_not_each_2nd, w_empty, W1, W2, b2, W3, b3):
    from concourse.bass_utils import run_bass_kernel_spmd

    consts = build_consts(w_each, w_not_each, w_not_each_2nd, w_empty,
                          W1, W2, b2, W3, b3)
    xms, xts = _prep_inputs(dots)

    nc = bacc.Bacc()
    emit_kernel(nc)
    nc.compile()

    in_maps = [dict(consts, xm=xms[k], xt4=xts[k]) for k in range(NCORES)]
    res = run_bass_kernel_spmd(nc, in_maps, list(range(NCORES)))
    out = np.concatenate([np.asarray(r["out"]).reshape(BC) for r in res.results])
    return out.reshape(B, 1).astype(np.float32)


if __name__ == "__main__":
    rng = np.random.default_rng(0)
    ins = {
        "dots": rng.integers(0, 6, size=(V, H, B)).astype(np.int32),
        "w_each": rng.standard_normal((FULL, CT), dtype=np.float32) * 0.1,
        "w_not_each": rng.standard_normal((FULL, CT), dtype=np.float32) * 0.1,
        "w_not_each_2nd": rng.standard_normal((FULL, CT), dtype=np.float32) * 0.1,
        "w_empty": rng.standard_normal((FULL, CT), dtype=np.float32) * 0.1,
        "W1": rng.standard_normal((NSQ, NPIX), dtype=np.float32) * 0.2,
        "W2": rng.standard_normal((NSQ, NSQ), dtype=np.float32) * 0.2,
        "b2": rng.standard_normal(NSQ, dtype=np.float32) * 0.1,
        "W3": rng.standard_normal((1, NSQ), dtype=np.float32) * 0.2,
        "b3": rng.standard_normal(1, dtype=np.float32) * 0.1,
    }
    y = kernel(**ins)
    print("kernel out", y.shape, y[:4, 0])
